# revision 1
# baseline (speedup 1.0000x reference)
"""Bass/Trainium2 kernel for nn_EnergyOutputCollector.

Math (per batch row b):
    w[c]      = position_weights.flat[cell_ids[c]]
    surface   = scatter(energy * w) -> [B, 1024]   (cell_ids is a permutation)
    h1 = LN(gelu_tanh(surface @ W1 + b1)) * g1 + bb1
    h2 = LN(gelu_tanh(h1 @ W2 + b2)) * g2 + bb2
    out = h2 @ W3 + b3

Strategy:
  - Data-parallel: batch (16384) split across 8 NeuronCores (2048 each).
  - The scatter + position-weight gather + LN affine params fold into the
    weights on the host in exact fp32:
        W1' = (w[:,None] * W1[cell_ids])          (scatter == row gather of W1)
        W2' = diag(g1) @ W2,  bias2' = bb1 @ W2 + b2
        W3' = diag(g2) @ W3,  bias3' = bb2 @ W3 + b3
  - Device: pure 3-layer MLP in fp16 (PE full rate, ~1e-3 rel err overall),
    fp32 PSUM accumulation, gelu on ScalarE straight from PSUM (one op per
    2048-wide layer output, spanning 4 PSUM banks), LayerNorm stats via
    bn_stats/bn_aggr on VectorE, one batched fp16 SBUF->SBUF DMA transpose
    per layer output (contraction dim must sit on partitions for the next
    matmul).
  - Batch tiles processed layer-major in groups of 4 so ScalarE activation
    ops of the same function (gelu vs sqrt table sets) batch together --
    ACT table-set swaps cost ~1.3-2.7us each.
"""

import numpy as np

import concourse.bass as bass
import concourse.mybir as mybir
import concourse.tile as tile
from concourse import bacc
from concourse.bass_utils import run_bass_kernel_spmd

N_CORES = 8
SURF = 1024
HID = 2048
INTER = 2048
OUT = 768
BATCH = 16384
BC = BATCH // N_CORES          # batch per core
MT = BC // 128                 # m-tiles per core (16)
GROUP = 4                      # m-tiles per layer-major group
EPS = 1e-5

F = mybir.ActivationFunctionType
ALU = mybir.AluOpType
F16 = mybir.dt.float16
F32 = mybir.dt.float32

_PROGRAM_CACHE: dict = {}
_LAST_EXEC_NS = None


def _build_program(with_b1: bool, with_b2: bool, with_b3: bool, repeats: int = 1,
                   ps_w: int = 2048, ps_bufs: int = 2, group: int = GROUP):
    assert MT % group == 0, (MT, group)
    nc = bacc.Bacc(None, target_bir_lowering=False, debug=False)

    e = nc.dram_tensor("e", [SURF, BC], F16, kind="ExternalInput")
    w1 = nc.dram_tensor("w1", [SURF, HID], F16, kind="ExternalInput")
    w2 = nc.dram_tensor("w2", [HID, INTER], F16, kind="ExternalInput")
    w3 = nc.dram_tensor("w3", [INTER, OUT], F16, kind="ExternalInput")
    b1d = nc.dram_tensor("b1", [HID], F16, kind="ExternalInput") if with_b1 else None
    b2d = nc.dram_tensor("b2", [INTER], F16, kind="ExternalInput") if with_b2 else None
    b3d = nc.dram_tensor("b3", [OUT], F32, kind="ExternalInput") if with_b3 else None
    out = nc.dram_tensor("out", [BC, OUT], F32, kind="ExternalOutput")

    # K-on-partitions layouts for matmul operands
    et = e.rearrange("(ko p) b -> p ko b", p=128)      # [128, 8, BC]
    w1t = w1.rearrange("(ko p) n -> p ko n", p=128)    # [128, 8, HID]
    w2t = w2.rearrange("(ko p) n -> p ko n", p=128)    # [128, 16, INTER]
    w3t = w3.rearrange("(ko p) n -> p ko n", p=128)    # [128, 16, OUT]

    with tile.TileContext(nc) as tc:
        with (
            tc.tile_pool(name="weights", bufs=1) as wp,
            tc.tile_pool(name="consts", bufs=1) as cp,
            tc.tile_pool(name="etiles", bufs=group + 2) as ep,
            tc.tile_pool(name="acts", bufs=group + 2) as hp,
            tc.tile_pool(name="actsT", bufs=group + 2) as tp,
            tc.tile_pool(name="stats", bufs=2 * group) as sp,
            tc.tile_pool(name="outs", bufs=min(group, 4)) as op,
            tc.tile_pool(name="psum", bufs=ps_bufs, space="PSUM") as pp,
        ):
            w1_sb = wp.tile([128, 8, HID], F16, tag="w1")
            w2_sb = wp.tile([128, 16, INTER], F16, tag="w2")
            w3_sb = wp.tile([128, 16, OUT], F16, tag="w3")
            nc.sync.dma_start(w1_sb[:], w1t)
            nc.sync.dma_start(w2_sb[:], w2t)
            nc.sync.dma_start(w3_sb[:], w3t)

            eps_t = cp.tile([128, 1], F32, tag="eps")
            nc.vector.memset(eps_t[:], EPS)

            def bias_bc(dram_vec, n, tag, dt=F32):
                t = cp.tile([128, n], dt, tag=tag)
                src = bass.AP(
                    tensor=dram_vec.tensor,
                    offset=dram_vec.offset,
                    ap=[[0, 128]] + list(dram_vec.ap),
                )
                nc.sync.dma_start(t[:], src)
                return t

            b1_sb = bias_bc(b1d[:], HID, "b1", F16) if with_b1 else None
            b2_sb = bias_bc(b2d[:], INTER, "b2", F16) if with_b2 else None
            b3_sb = bias_bc(b3d[:], OUT, "b3") if with_b3 else None

            def mlp_layer(lhsT_of, w_sb, nk, bias_sb, tiles):
                """matmul over nk k-chunks + optional bias + gelu, for each
                tile in `tiles`; returns gelu'd fp16 [128, 2048] tiles."""
                hs = {}
                for t in tiles:
                    h = hp.tile([128, 2048], F16, tag="h")
                    for p0 in range(0, 2048, ps_w):
                        ps = pp.tile([128, ps_w], F32, tag="ps")
                        for n in range(ps_w // 512):
                            psl = slice(n * 512, (n + 1) * 512)
                            nsl = slice(p0 + n * 512, p0 + (n + 1) * 512)
                            for k in range(nk):
                                nc.tensor.matmul(
                                    ps[:, psl], lhsT_of(t, k), w_sb[:, k, nsl],
                                    start=(k == 0), stop=(k == nk - 1),
                                )
                        hsl = slice(p0, p0 + ps_w)
                        if bias_sb is not None:
                            nc.vector.tensor_add(
                                out=ps[:], in0=ps[:], in1=bias_sb[:, hsl]
                            )
                        nc.scalar.activation(
                            out=h[:, hsl], in_=ps[:], func=F.Gelu_apprx_tanh
                        )
                    hs[t] = h
                return hs

            def layernorm_transpose(hs, tiles):
                """LN (in place) then batched transpose; returns hT tiles."""
                mvs = {}
                for t in tiles:
                    stats = sp.tile([128, 4, 6], F32, tag="stats")
                    for n in range(4):
                        nc.vector.bn_stats(
                            out=stats[:, n, :], in_=hs[t][:, n * 512:(n + 1) * 512]
                        )
                    mv = sp.tile([128, 2], F32, tag="mv")
                    nc.vector.bn_aggr(out=mv[:], in_=stats[:])
                    mvs[t] = mv
                rs = {}
                for t in tiles:   # batched: one ACT table-set swap per group
                    r = sp.tile([128, 1], F32, tag="rstd")
                    nc.scalar.activation(
                        out=r[:], in_=mvs[t][:, 1:2], func=F.Sqrt, bias=eps_t[:]
                    )
                    rs[t] = r
                hts = {}
                for t in tiles:
                    nc.vector.reciprocal(out=rs[t][:], in_=rs[t][:])
                    nc.vector.tensor_scalar(
                        out=hs[t][:], in0=hs[t][:],
                        scalar1=mvs[t][:, 0:1], scalar2=rs[t][:],
                        op0=ALU.subtract, op1=ALU.mult,
                    )
                    ht = tp.tile([128, 16, 128], F16, tag="ht")
                    nc.sync.dma_start_transpose(ht[:], hs[t][:])
                    hts[t] = ht
                return hts

            def _full_body():
                for g in range(MT // group):
                    tiles = list(range(g * group, (g + 1) * group))

                    e_sbs = {}
                    for t in tiles:
                        e_sb = ep.tile([128, 8, 128], F16, tag="e")
                        nc.sync.dma_start(
                            e_sb[:], et[:, :, t * 128:(t + 1) * 128]
                        )
                        e_sbs[t] = e_sb

                    h1 = mlp_layer(
                        lambda t, k: e_sbs[t][:, k, :], w1_sb, 8, b1_sb, tiles
                    )
                    h1T = layernorm_transpose(h1, tiles)
                    h2 = mlp_layer(
                        lambda t, k: h1T[t][:, k, :], w2_sb, 16, b2_sb, tiles
                    )
                    h2T = layernorm_transpose(h2, tiles)

                    for t in tiles:
                        if ps_w >= 1024:
                            ps = pp.tile([128, ps_w], F32, tag="ps", name="ps_l3")
                        else:
                            ps = pp.tile([128, 1024], F32, tag="ps3", name="ps_l3")
                        for n in range(2):
                            bsl = slice(n * 512, n * 512 + 384)
                            nsl = slice(n * 384, (n + 1) * 384)
                            for k in range(16):
                                nc.tensor.matmul(
                                    ps[:, bsl], h2T[t][:, k, :], w3_sb[:, k, nsl],
                                    start=(k == 0), stop=(k == 15),
                                )
                        ps3 = ps[:, :1024].rearrange("p (b f) -> p b f", f=512)[:, :2, :384]
                        o_sb = op.tile([128, 2, 384], F32, tag="o")
                        if b3_sb is not None:
                            nc.vector.tensor_tensor(
                                o_sb[:], ps3,
                                b3_sb[:].rearrange("p (b f) -> p b f", f=384),
                                ALU.add,
                            )
                        else:
                            nc.vector.tensor_copy(o_sb[:], ps3)
                        nc.sync.dma_start(
                            out[t * 128:(t + 1) * 128, :],
                            o_sb[:].rearrange("p b f -> p (b f)"),
                        )

            if repeats == 1:
                _full_body()
            else:
                with tc.For_i(0, repeats, 1):
                    _full_body()

    nc.compile()
    return nc


def _prepare(energy, cell_ids, position_weights, W1, b1, ln1_g, ln1_b,
             W2, b2, ln2_g, ln2_b, W3, b3):
    """Host-side prep: shard + fold scatter/gather/LN-affine into weights.
    Returns (bias_flags_key, per-core input maps)."""
    energy = np.asarray(energy, dtype=np.float32)
    cell_ids = np.asarray(cell_ids)
    position_weights = np.asarray(position_weights, dtype=np.float32)
    W1 = np.asarray(W1, dtype=np.float32)
    W2 = np.asarray(W2, dtype=np.float32)
    W3 = np.asarray(W3, dtype=np.float32)
    b1 = np.asarray(b1, dtype=np.float32)
    b2 = np.asarray(b2, dtype=np.float32)
    b3 = np.asarray(b3, dtype=np.float32)
    ln1_g = np.asarray(ln1_g, dtype=np.float32)
    ln1_b = np.asarray(ln1_b, dtype=np.float32)
    ln2_g = np.asarray(ln2_g, dtype=np.float32)
    ln2_b = np.asarray(ln2_b, dtype=np.float32)

    ids = cell_ids.astype(np.int64)
    # scatter surface[:, ids] = (energy * w).T  ==  row-gather of W1 at ids
    # (ids is a permutation: fill=arange per the problem spec)
    w = position_weights.reshape(-1)[ids]
    W1f = w[:, None] * W1[ids]

    # fold LN affine params into the next layer (exact fp32 host math):
    # (xn*g + lb) @ W + b  ==  xn @ (diag(g) W) + (lb @ W + b)
    W2f = ln1_g[:, None] * W2
    b2f = ln1_b @ W2 + b2
    W3f = ln2_g[:, None] * W3
    b3f = ln2_b @ W3 + b3

    with_b1 = bool(np.any(b1 != 0.0))
    with_b2 = bool(np.any(b2f != 0.0))
    with_b3 = bool(np.any(b3f != 0.0))
    key = (with_b1, with_b2, with_b3)

    base = {
        "w1": W1f.astype(np.float16),
        "w2": W2f.astype(np.float16),
        "w3": W3f.astype(np.float16),
    }
    if with_b1:
        base["b1"] = b1.astype(np.float16)
    if with_b2:
        base["b2"] = b2f.astype(np.float16)
    if with_b3:
        base["b3"] = b3f

    e16 = energy.astype(np.float16)
    in_maps = [
        {**base, "e": np.ascontiguousarray(e16[:, c * BC:(c + 1) * BC])}
        for c in range(N_CORES)
    ]
    return key, in_maps


def kernel(energy, cell_ids, position_weights, W1, b1, ln1_g, ln1_b,
           W2, b2, ln2_g, ln2_b, W3, b3):
    key, in_maps = _prepare(energy, cell_ids, position_weights, W1, b1,
                            ln1_g, ln1_b, W2, b2, ln2_g, ln2_b, W3, b3)
    if key not in _PROGRAM_CACHE:
        _PROGRAM_CACHE[key] = _build_program(*key)
    nc = _PROGRAM_CACHE[key]
    res = run_bass_kernel_spmd(nc, in_maps, core_ids=list(range(N_CORES)))
    global _LAST_EXEC_NS
    if res.exec_time_ns is not None:
        _LAST_EXEC_NS = res.exec_time_ns
    return np.concatenate([r["out"] for r in res.results], axis=0)

